# revision 36
# baseline (speedup 1.0000x reference)
"""Trainium2 Bass kernel for nn_DeterministicEncoder (MLP encoder + Laplace-kernel
attention), sorted-window formulation.

  ctx = [x_context, y_context]            # [M, 2]
  h   = relu(ctx @ W1 + b1); h = relu(h @ W2 + b2); v = h @ W3 (+ b3=0)
  out[n] = sum_m exp(-|k_m - q_n|) * v[m],  k = x_context, q = x_target

Factorized attention: exp(-|k-q|) = [k<=q] e^k e^-q + [k>q] e^-k e^q, so with
av = e^k v, bv = e^-k v:
  out[n] = e^-q_n * P1(n) + e^q_n * (SB - P2(n)),
  P1(n) = sum_{k_m<=q_n} av[m], P2(n) = sum_{k_m<=q_n} bv[m], SB = sum_m bv[m].

Sharding = value-range: sort k and q on the host (pure index permutation), give
core c a contiguous run of 1024 sorted targets. All k's straddling that q-range
then live in a 12-tile (1536-context) window of the sorted k array; the host
rotates each core's context layout so its window is always local tiles 0..11.
Per core:
  P1(n) = C1 + sum_{m in window, k_m<=q_n} av[m]          (C1 = av-sum, tiles < t0)
  SB - P2(n) = C2 - sum_{m in window, k_m<=q_n} bv[m]     (C2 = bv-sum, tiles >= t0)
C1/C2 come from per-tile partial sums (segmented reduce of scaled h2, then a
tiny matmul through W3 and a host-supplied 0/1 tile-selection mask), so the
masked matmuls run over 12 of 64 m-tiles instead of all of them.

Host prep is sorting/relayout + dtype casts only; all FLOPs run on device.
"""

import numpy as np
import ml_dtypes

import concourse.bass as bass
import concourse.tile as tile
from concourse import mybir
from concourse.bass_utils import run_bass_kernel_spmd

N_CORES = 8
FAST = True  # kept for test.py compatibility; kernel is always bf16-attention
M = 8192
N = 8192
N_SH = N // N_CORES  # 1024 targets per core
H = 16
OUT = 64
NG = 8               # m-groups stacked across partitions in the MLP
MJ = M // NG         # 1024 contexts per group
WT = 12              # window tiles (128 contexts each) per core
FB = 6               # m-tiles per mask batch (2 batches x 2 bufs -> no reuse)

F32 = mybir.dt.float32
F32R = mybir.dt.float32r
BF16 = mybir.dt.bfloat16

# ctx_d columns (16 partitions)
CW_CTX = 0                  # ctxs f32 [16, MJ] (exp/broadcast source)
CW_CTXB = CW_CTX + MJ       # ctxs bf16 pairs [16, 512] (matmul rhs)
CW_W1 = CW_CTXB + 512       # w1 block-diag bf16 pairs [16, 64]
CW_SEL = CW_W1 + 64         # SEL mask, av (bv rows zeroed) [16, 512]
CW_SELB = CW_SEL + 512      # SEL mask, bv (av rows zeroed) [16, 512]
CW_ONE = CW_SELB + 512      # ones column [16, 1]
CW = CW_ONE + 1

# blob_d columns (128 partitions)
BW_W2 = 0                   # w2 block-diag bf16 pairs [128, 64]
BW_W3 = BW_W2 + 64          # w3 stacked bf16 pairs [128, 256]
BW_B1 = BW_W3 + 256
BW_B2 = BW_B1 + 1
BW_SGN = BW_B2 + 1          # -1 rows 0-63, +1 rows 64-127
BW_KW = BW_SGN + 1          # window k columns f32 [128, WT]
BW = BW_KW + WT


def _build(legalize: bool = True) -> bass.Bass:
    nc = bass.Bass()
    ctx_d = nc.dram_tensor("ctxb", [16, CW], F32, kind="ExternalInput")
    q_d = nc.dram_tensor("qrow", [1, N_SH + N_SH // 2], F32, kind="ExternalInput")
    blob_d = nc.dram_tensor("blob", [128, BW], F32, kind="ExternalInput")
    xr_d = nc.dram_tensor("xr", [128, MJ], F32, kind="ExternalInput")
    out_d = nc.dram_tensor("out", [128, N_SH], F32, kind="ExternalOutput")

    with tile.TileContext(nc) as tc:
        with (
            tc.tile_pool(name="const", bufs=1) as const,
            tc.tile_pool(name="mbuf", bufs=2) as mbuf,
            tc.tile_pool(name="mlpps", bufs=1, space="PSUM") as mlpps,
            tc.tile_pool(name="vps", bufs=2, space="PSUM") as vps,
            tc.tile_pool(name="ops", bufs=1, space="PSUM") as ops,
        ):
            cs = const.tile([16, CW], F32)
            nc.sync.dma_start(out=cs[:], in_=ctx_d[:, :])
            blob = const.tile([128, BW], F32)
            nc.sync.dma_start(out=blob[:], in_=blob_d[:, :])

            ctxs = cs[0:16, CW_CTX:CW_CTX + MJ]
            ctxs16 = cs[0:16, CW_CTXB:CW_CTXB + 512].bitcast(BF16)
            w1 = cs[0:16, CW_W1:CW_W1 + 64].bitcast(BF16)
            selmA = cs[0:16, CW_SEL:CW_SEL + 512]
            selmB = cs[0:16, CW_SELB:CW_SELB + 512]
            ones1 = cs[0:16, CW_ONE:CW_ONE + 1]
            w2 = blob[:, BW_W2:BW_W2 + 64].bitcast(BF16)
            w3 = blob[:, BW_W3:BW_W3 + 256].bitcast(BF16)   # [128, 512] bf16
            b1 = blob[:, BW_B1:BW_B1 + 1]
            b2 = blob[:, BW_B2:BW_B2 + 1]
            sgn = blob[:, BW_SGN:BW_SGN + 1]
            kw = blob[:, BW_KW:BW_KW + WT]

            # early per-engine consumes of the input DMAs: walrus allows only
            # ONE cross-engine wait per compute instruction, so each engine
            # touches each DMA'd tile once up front to absorb that semaphore.
            tch = const.tile([128, 6], F32)
            tch16 = const.tile([16, 2], F32)
            nc.scalar.copy(tch[:, 0:1], blob[:, 0:1])
            nc.vector.tensor_copy(tch[:, 1:2], blob[:, 0:1])
            nc.vector.tensor_copy(tch16[:, 0:1], cs[0:16, 0:1])
            nc.gpsimd.tensor_copy(tch[:, 2:3], blob[:, 0:1])

            # ---- q broadcasts via stride-0 DMA (re-read one HBM row 128x)
            qb = const.tile([128, N_SH], F32)
            qsrc = q_d[0:1, 0:N_SH]
            qsrc_b, _ = bass.broadcast_tensor_aps(qsrc, qb[:])
            nc.sync.dma_start(out=qb[:], in_=qsrc_b)
            qm = const.tile([128, N_SH], BF16)
            qsrc16 = q_d[0:1, N_SH:N_SH + N_SH // 2].bitcast(BF16)
            qsrc16_b, _ = bass.broadcast_tensor_aps(qsrc16, qm[:])
            nc.sync.dma_start(out=qm[:], in_=qsrc16_b)
            xrep = const.tile([128, MJ], F32)
            nc.sync.dma_start(out=xrep[:], in_=xr_d[:, :])
            # DVE consume of gpsimd's qm (before the DVE-issued masks)
            qmt = const.tile([128, 1], BF16)
            nc.vector.tensor_copy(qmt[:], qm[:, 0:1])

            # PE consume of the blob DMA (w2/w3 are PE's first blob reads)
            scr0 = ops.tile([128, 1], F32, tag="c")
            nc.tensor.matmul(scr0[0:1, 0:1], blob[0:1, 0:1], blob[0:1, 0:1],
                             start=True, stop=True)

            # ---- MLP layers 1/2 (bf16 matmul, relu+bias on ACT)
            ps1 = mlpps.tile([128, MJ], F32, tag="ps")
            for c in range(MJ // 512):
                nc.tensor.matmul(ps1[:, bass.ts(c, 512)], w1,
                                 ctxs16[:, bass.ts(c, 512)],
                                 start=True, stop=True)
            h1 = const.tile([128, MJ], BF16)
            nc.scalar.activation(h1[:], ps1[:],
                                 mybir.ActivationFunctionType.Relu, bias=b1)
            ps2 = mlpps.tile([128, MJ], F32, tag="ps")
            for c in range(MJ // 512):
                nc.tensor.matmul(ps2[:, bass.ts(c, 512)], w2,
                                 h1[:, bass.ts(c, 512)],
                                 start=True, stop=True)
            h2 = const.tile([128, MJ], F32)
            nc.scalar.activation(h2[:], ps2[:],
                                 mybir.ActivationFunctionType.Relu, bias=b2)

            # ---- column scales e^{+-k} (ACT) and scaled copies (DVE/gpsimd)
            A1 = const.tile([128, MJ], F32)
            nc.scalar.activation(A1[:], xrep[:],
                                 mybir.ActivationFunctionType.Exp, scale=1.0)
            A2 = const.tile([128, MJ], F32)
            nc.scalar.activation(A2[:], xrep[:],
                                 mybir.ActivationFunctionType.Exp, scale=-1.0)
            h2a = const.tile([128, MJ], BF16)
            nc.vector.tensor_mul(h2a[:], h2[:], A1[:])
            h2b = const.tile([128, MJ], BF16)
            nc.gpsimd.tensor_mul(h2b[:], h2[:], A2[:])

            # ---- row factors: bq2 = exp(sgn * q), sgn = -1 top / +1 bottom
            bq2 = const.tile([128, N_SH], F32)
            nc.scalar.activation(bq2[:], qb[:],
                                 mybir.ActivationFunctionType.Exp, scale=sgn)

            # ---- per-tile sums -> C1/C2 prefix constants
            Sab = const.tile([128, 16], F32)
            nc.vector.tensor_reduce(
                Sab[:, 0:8], h2a[:].rearrange("p (j i) -> p j i", i=128),
                axis=mybir.AxisListType.X, op=mybir.AluOpType.add)
            nc.vector.tensor_reduce(
                Sab[:, 8:16], h2b[:].rearrange("p (j i) -> p j i", i=128),
                axis=mybir.AxisListType.X, op=mybir.AluOpType.add)
            Sab16 = const.tile([128, 16], BF16)
            nc.vector.tensor_copy(Sab16[:], Sab[:])
            St = mlpps.tile([16, 512], F32, tag="st")
            nc.tensor.matmul(St[:], Sab16[:], w3[:], start=True, stop=True)
            mskA = const.tile([16, 512], F32)
            nc.vector.tensor_mul(mskA[:], St[:], selmA)
            mskB = const.tile([16, 512], F32)
            nc.vector.tensor_mul(mskB[:], St[:], selmB)
            # CpartX cols 0-63 <- av sums (bv rows land as 0 via selmA), cols
            # 64-127 <- bv sums; then one 1-col matmul collapses partitions.
            CpartX = const.tile([16, 128], F32)
            nc.vector.tensor_reduce(
                CpartX[:, 0:64], mskA[:].rearrange("p (g o) -> p o g", o=64),
                axis=mybir.AxisListType.X, op=mybir.AluOpType.add)
            nc.vector.tensor_reduce(
                CpartX[:, 64:128], mskB[:].rearrange("p (g o) -> p o g", o=64),
                axis=mybir.AxisListType.X, op=mybir.AluOpType.add)
            Cps = ops.tile([128, 1], F32, tag="c")
            nc.tensor.matmul(Cps[:], CpartX[:], ones1, start=True, stop=True)
            Ccol = const.tile([128, 1], F32)
            nc.scalar.copy(Ccol[:], Cps[:])

            # ---- window v-tiles: packed lhsT per tile t: [av(64) | bv(64)]
            # local m = g*MJ + jt*128 + p ; local tile t = g*8 + jt < WT
            v_sb = const.tile([128, WT * 128], BF16)
            v_vw = v_sb[:].rearrange("p (t c) -> p t c", c=128)
            for half, h2x in ((0, h2a), (1, h2b)):
                for jt in range(8):
                    ngr = 2 if jt < WT - 8 else 1
                    pv = vps.tile([128, 64 * ngr], F32)
                    nc.tensor.matmul(pv[:], h2x[:, bass.ts(jt, 128)],
                                     w3[:, 0:64 * ngr], start=True, stop=True)
                    dst = v_vw[:, jt:WT:8, half * 64:(half + 1) * 64]
                    src = pv[:].rearrange("p (g c) -> p g c", c=64)
                    if half == 0:
                        nc.scalar.copy(dst, src)
                    else:
                        nc.vector.tensor_copy(dst, src)

            # PE pre-consumes of v_sb (one per writer engine: ACT av-half,
            # gpsimd bv-half) so the first attention matmul only waits on DVE
            scr = ops.tile([128, 1], F32, tag="c")
            nc.tensor.matmul(scr[0:1, 0:1], v_sb[0:1, 0:1], v_sb[0:1, 0:1],
                             start=True, stop=True)
            nc.tensor.matmul(scr[0:1, 0:1], v_sb[0:1, 64:65], v_sb[0:1, 64:65],
                             start=True, stop=True)

            # ---- attention over the window: psum rows 0-63 = P1, 64-127 = P2
            po = ops.tile([128, N_SH], F32, tag="po")
            for b in range(WT // FB):
                m = mbuf.tile([128, FB * N_SH], BF16)
                for i in range(FB):
                    t = b * FB + i
                    eng = nc.vector if t % 3 else nc.gpsimd
                    eng.tensor_scalar(
                        out=m[:, i * N_SH:(i + 1) * N_SH], in0=qm[:],
                        scalar1=kw[:, t:t + 1], scalar2=None,
                        op0=mybir.AluOpType.is_ge)
                for i in range(FB):
                    t = b * FB + i
                    for u in range(N_SH // 512):
                        nc.tensor.matmul(
                            po[:, bass.ts(u, 512)], v_vw[:, t:t + 1, :],
                            m[:, i * N_SH + u * 512:i * N_SH + (u + 1) * 512],
                            start=(t == 0), stop=(t == WT - 1))

            # DVE/gpsimd consumes of ACT's Ccol (covers bq2 too — same
            # semaphore, later tick) so the epilogue ops only wait on PE
            nc.vector.tensor_copy(tch[:, 3:4], Ccol[:])
            nc.gpsimd.tensor_copy(tch[:, 4:5], Ccol[:])

            # ---- epilogue: top = (P1 + C1) * e^-q ; bot = (P2 - C2) * e^q
            # host computes out = (top - bot).T
            obuf = const.tile([128, N_SH], F32)
            nc.vector.scalar_tensor_tensor(
                out=obuf[0:OUT, :], in0=po[0:OUT, :], scalar=Ccol[0:OUT, :],
                in1=bq2[0:OUT, :], op0=mybir.AluOpType.add,
                op1=mybir.AluOpType.mult)
            nc.vector.scalar_tensor_tensor(
                out=obuf[OUT:128, :], in0=po[OUT:128, :],
                scalar=Ccol[OUT:128, :], in1=bq2[OUT:128, :],
                op0=mybir.AluOpType.subtract, op1=mybir.AluOpType.mult)
            nc.sync.dma_start(out=out_d[:, :], in_=obuf[:])

    if legalize:
        _fix_tsp_waits(nc)
    return nc


def _fix_tsp_waits(nc: bass.Bass) -> None:
    """Walrus accepts at most ONE sync-wait per compute instruction (and few
    on the tail drain). Same-engine self-waits are redundant — every engine
    completes its queue strictly in order — so drop them; the tail drain
    keeps only the output-DMA wait (the dag funnels through it)."""
    budget = {"InstTensorScalarPtr": 1, "InstMatmult": 1, "InstTensorCopy": 1,
              "InstMemset": 1, "InstActivation": 1, "InstTensorTensor": 1,
              "InstScalarTensorTensor": 1, "InstTensorReduce": 1,
              "InstPartitionBroadcast": 1, "InstTensorScalar": 1}
    eng_prefix = {"DVE": "DVE_", "Activation": "Activation_", "PE": "PE_",
                  "SP": "SP_", "Pool": "Pool_"}
    blocks = nc.m.functions[0].blocks
    out_dma_sems: set[str] = set()
    for b in blocks:
        for inst in b.instructions:
            if type(inst).__name__ == "InstDMACopy" and inst.sync_info:
                writes_out = any("out" == getattr(x, "memref", None)
                                 for x in inst.outs)
                if writes_out:
                    out_dma_sems |= {u.ant_name for u in inst.sync_info.on_update
                                     if u.ant_name}
    for b in blocks:
        for inst in b.instructions:
            tname = type(inst).__name__
            si = inst.sync_info
            if si is None:
                continue
            if tname == "InstDrain" and len(si.on_wait) > 2:
                kept = [w for w in si.on_wait if w.ant_name in out_dma_sems]
                if not 1 <= len(kept) <= 2:
                    raise RuntimeError(f"tail drain {inst.name}: waits "
                                       f"{[(w.ant_name, w.wait_value) for w in si.on_wait]}")
                si.on_wait = kept
                inst.sync_info = si
                continue
            lim = budget.get(tname)
            if lim is None or len(si.on_wait) <= lim:
                continue
            eng = str(inst.engine).split(".")[-1]
            pfx = eng_prefix.get(eng, "\x00")
            kept = [w for w in si.on_wait
                    if not (w.ant_name or "").startswith(pfx)]
            if len(kept) > lim:
                raise RuntimeError(
                    f"{inst.name} ({tname}, {eng}): "
                    f"{[(w.ant_name, w.wait_value) for w in si.on_wait]}")
            si.on_wait = kept
            inst.sync_info = si


def _prep_maps(inputs: dict) -> tuple[list[dict], np.ndarray]:
    xc = np.ascontiguousarray(inputs["x_context"], dtype=np.float32).reshape(M)
    yc = np.ascontiguousarray(inputs["y_context"], dtype=np.float32).reshape(M)
    xt = np.ascontiguousarray(inputs["x_target"], dtype=np.float32).reshape(N)
    W1 = np.asarray(inputs["W1"], dtype=np.float32)
    b1 = np.asarray(inputs["b1"], dtype=np.float32)
    W2 = np.asarray(inputs["W2"], dtype=np.float32)
    b2 = np.asarray(inputs["b2"], dtype=np.float32)
    W3 = np.asarray(inputs["W3"], dtype=np.float32)

    pk = np.argsort(xc, kind="stable")
    ks, ys = xc[pk], yc[pk]
    pq = np.argsort(xt, kind="stable")
    qs = xt[pq]

    w1bd = np.zeros((16, 128), dtype=np.float32)
    w2bd = np.zeros((128, 128), dtype=np.float32)
    w3stk = np.zeros((128, NG * OUT), dtype=np.float32)
    for g in range(NG):
        w1bd[2 * g:2 * g + 2, H * g:H * (g + 1)] = W1
        w2bd[H * g:H * (g + 1), H * g:H * (g + 1)] = W2
        w3stk[H * g:H * (g + 1), OUT * g:OUT * (g + 1)] = W3
    as16 = lambda a: np.ascontiguousarray(
        a.astype(ml_dtypes.bfloat16)).view(np.float32)
    w1f32 = as16(w1bd)
    w2f32 = as16(w2bd)
    w3f32 = as16(w3stk)
    b1s = np.tile(b1, NG).astype(np.float32)
    b2s = np.tile(b2, NG).astype(np.float32)

    maps = []
    for c in range(N_CORES):
        q = qs[c * N_SH:(c + 1) * N_SH]
        r_lo = int(np.searchsorted(ks, q[0], side="left"))
        r_hi = int(np.searchsorted(ks, q[-1], side="right"))
        t0 = max(0, min(r_lo // 128, 64 - WT))
        if r_hi > 128 * (t0 + WT):
            raise RuntimeError(
                f"core {c}: rank span [{r_lo}, {r_hi}) exceeds {WT}-tile window")
        rot = np.roll(np.arange(M), -128 * t0)
        kl, yl = ks[rot], ys[rot]

        ctxb = np.zeros((16, CW), dtype=np.float32)
        ctxb[0::2, CW_CTX:CW_CTX + MJ] = kl.reshape(NG, MJ)
        ctxb[1::2, CW_CTX:CW_CTX + MJ] = yl.reshape(NG, MJ)
        ctxb[:, CW_CTXB:CW_CTXB + 512] = as16(ctxb[:, CW_CTX:CW_CTX + MJ])
        ctxb[:, CW_W1:CW_W1 + 64] = w1f32
        # SEL rows 0-7 (av, by jt): local tile T=g*8+jt is global (t0+T)%64;
        # av selects global tiles < t0 <=> T >= 64-t0; bv the complement.
        tloc = np.arange(64).reshape(NG, 8)            # [g, jt]
        sel_av = (tloc >= 64 - t0).astype(np.float32) if t0 else np.zeros((NG, 8), np.float32)
        sel_bv = 1.0 - sel_av
        # selmA: av selection on rows 0-7 (bv rows zero); selmB: bv on 8-15
        ctxb[0:8, CW_SEL:CW_SEL + 512] = np.repeat(sel_av.T, OUT, axis=1)
        ctxb[8:16, CW_SELB:CW_SELB + 512] = np.repeat(sel_bv.T, OUT, axis=1)
        ctxb[:, CW_ONE] = 1.0

        qrow = np.zeros((1, N_SH + N_SH // 2), dtype=np.float32)
        qrow[0, 0:N_SH] = q
        qrow[0, N_SH:] = np.ascontiguousarray(
            q.astype(ml_dtypes.bfloat16)).view(np.float32)
        xrep = np.repeat(kl.reshape(NG, MJ), H, axis=0)     # [128, MJ]

        blob = np.zeros((128, BW), dtype=np.float32)
        blob[:, BW_W2:BW_W2 + 64] = w2f32
        blob[:, BW_W3:BW_W3 + 256] = w3f32
        blob[:, BW_B1] = b1s
        blob[:, BW_B2] = b2s
        blob[0:OUT, BW_SGN] = -1.0
        blob[OUT:128, BW_SGN] = 1.0
        blob[:, BW_KW:BW_KW + WT] = kl[:WT * 128].reshape(WT, 128).T
        maps.append({"ctxb": ctxb, "qrow": qrow, "blob": blob, "xr": xrep})
    return maps, pq


def _unshard(results: list[dict], pq: np.ndarray, b3: np.ndarray) -> np.ndarray:
    if np.any(np.asarray(b3)):
        raise RuntimeError("nonzero b3 unsupported by the packed kernel")
    out = np.empty((N, OUT), dtype=np.float32)
    for c in range(N_CORES):
        o = results[c]["out"]                       # [128, N_SH]
        out[pq[c * N_SH:(c + 1) * N_SH]] = (o[:OUT] - o[OUT:]).T
    return out


def run(inputs: dict, fast: bool = True, **spmd_kwargs):
    nc = _build()
    in_maps, pq = _prep_maps(inputs)
    res = run_bass_kernel_spmd(nc, in_maps, list(range(N_CORES)), **spmd_kwargs)
    return _unshard(res.results, pq, inputs["b3"]), res


def kernel(**inputs) -> np.ndarray:
    out, _ = run(inputs)
    return out
